# revision 13
# baseline (speedup 1.0000x reference)
"""Trainium2 Bass kernel for the DCT-domain speech-enhancement DNN.

Reference computation (B=16 rows of L=160000 samples):
  - frame with WIN=320 / HOP=160 + hann window
  - DCT-II (keep NC=100 coeffs), clip to [-1,1]    -> noisy_dct, clean_dct
  - 3-layer 100x100 MLP (prelu, prelu, tanh)       -> output_dct
  - inverse DCT + overlap-add                      -> out_speech

Since HOP == WIN/2, framing and overlap-add reduce to dense matmuls over the
[1000, 160] hop-reshape of each signal plus a one-column shift that folds into
PSUM accumulation:
    dct[t]   = clip( H[t] @ Aw + H[t+1] @ Bw )          (Aw/Bw = win*Dc halves)
    speech_chunk[c] = od[c] @ DcT[:, :160] + od[c-1] @ DcT[:, 160:]
No gathers or scatter-adds are needed on device.

Sharding: pure data parallel, 2 batch rows per core across 8 cores; the small
MLP/DCT matrices are replicated.

Walrus limits how many sync waits a PE instruction can carry (transpose-mode
matmuls take only one), so the program is arranged so every PE instruction
depends on at most one "new" semaphore tick:
  - all constants arrive in ONE DMA; a dummy transpose pre-syncs PE on it
  - all PSUM->SBUF transpose copies run on DVE only
  - SBUF memsets run on GPSIMD; a second dummy transpose pre-syncs PE on them
  - per stream, transposes for 4 chunks batch into one PSUM bank, and the
    stream's DCT matmuls sync PE on DVE before the next stream reuses slots
"""

import sys

import numpy as np

sys.path.insert(0, "/opt/trn_rl_repo")

import concourse.bass as bass  # noqa: E402
import concourse.bacc as bacc  # noqa: E402
import concourse.tile as tile  # noqa: E402
from concourse import mybir  # noqa: E402
from concourse.bass_utils import run_bass_kernel_spmd  # noqa: E402

WIN, HOP, NC = 320, 160, 100
B, L = 16, 160000
T = (L - WIN) // HOP + 1  # 999 frames
NCH = L // HOP  # 1000 hop-chunks
NCORES = 8
RPC = B // NCORES  # rows per core
F32 = mybir.dt.float32

TCH = 125  # time-chunk for transposes / idct (8 * 125 = 1000)
NTCH = NCH // TCH

# const_pack column offsets
_CHI0 = 0  # [128, 200]   Aw/Bw rows 0:128
_CLO0 = 200  # [32, 200]  Aw/Bw rows 128:160
_WPK0 = 400  # [101, 300] W1T|W2T|W3T with bias row 100
_DCT0 = 700  # [100, 320] Dc.T
_ID0 = 1020  # [128, 128] identity
_CPW = 1148


def _host_constants():
    """DCT / window derived matrices, computed on host in float64 then cast."""
    n = np.arange(WIN, dtype=np.float64)
    win = 0.5 * (1.0 - np.cos(2.0 * np.pi * n / WIN))
    k = np.arange(WIN, dtype=np.float64)[None, :]
    ang = (np.arange(WIN, dtype=np.float64)[:, None] + 0.5) * np.pi / WIN
    D = np.sqrt(2.0 / WIN) * np.cos(ang @ k)
    D[:, 0] *= np.sqrt(0.5)
    Dc = D[:, :NC]  # [320, 100]
    Aw = (win[:HOP, None] * Dc[:HOP]).astype(np.float32)  # [160, 100]
    Bw = (win[HOP:, None] * Dc[HOP:]).astype(np.float32)  # [160, 100]
    chi = np.concatenate([Aw[:128], Bw[:128]], axis=1)  # [128, 200]
    clo = np.concatenate([Aw[128:], Bw[128:]], axis=1)  # [32, 200]
    dctT = np.ascontiguousarray(Dc.T).astype(np.float32)  # [100, 320]
    return chi, clo, dctT


def _const_pack(chi, clo, dctT, wpk):
    cp = np.zeros((128, _CPW), np.float32)
    cp[:, _CHI0 : _CHI0 + 200] = chi
    cp[:32, _CLO0 : _CLO0 + 200] = clo
    cp[: NC + 1, _WPK0 : _WPK0 + 300] = wpk
    cp[:NC, _DCT0 : _DCT0 + 320] = dctT
    cp[:, _ID0 : _ID0 + 128] = np.eye(128, dtype=np.float32)
    return cp


def _build(p1: float, p2: float):
    """Build the per-core Bass module (same program on all 8 cores)."""
    # Bacc's compile pipeline splits multi-waits into event-semaphore chains
    # (HW allows one sync wait per instruction)
    nc = bacc.Bacc()

    x_n = nc.dram_tensor("noisy", [RPC, NCH, HOP], F32, kind="ExternalInput")
    x_c = nc.dram_tensor("clean", [RPC, NCH, HOP], F32, kind="ExternalInput")
    cp_d = nc.dram_tensor("cpack", [128, _CPW], F32, kind="ExternalInput")

    od_out = nc.dram_tensor("od", [RPC, NC, T], F32, kind="ExternalOutput")
    cd_out = nc.dram_tensor("cd", [RPC, NC, T], F32, kind="ExternalOutput")
    sp_out = nc.dram_tensor("sp", [RPC, NCH, HOP], F32, kind="ExternalOutput")

    # prelu(x, p) = max(x, p*x) for p <= 1, min(x, p*x) for p >= 1
    pre_op = [
        mybir.AluOpType.max if p <= 1.0 else mybir.AluOpType.min for p in (p1, p2)
    ]

    with tile.TileContext(nc) as tc:
        with (
            tc.tile_pool(name="const", bufs=1) as cpool,
            tc.tile_pool(name="persist", bufs=1) as pers,
            tc.tile_pool(name="inp", bufs=16) as inpool,
            tc.tile_pool(name="evict", bufs=4) as evpool,
            tc.tile_pool(name="spout", bufs=16) as sppool,
            tc.tile_pool(name="ptr", bufs=2, space="PSUM") as ptr,
            tc.tile_pool(name="pmm", bufs=2, space="PSUM") as pmm,
            tc.tile_pool(name="po", bufs=2, space="PSUM") as po,
        ):
            # ---- constants: ONE DMA -> one semaphore for PE to sync on ----
            cp = cpool.tile([128, _CPW], F32)
            nc.sync.dma_start(cp[:], cp_d[:])
            ident = cp[:, _ID0 : _ID0 + 128]

            # pre-sync PE on the constants' DMA queue (1 wait)
            pdum = ptr.tile([32, 500], F32, tag="trl", name="pdum")
            nc.tensor.transpose(
                pdum[0:32, 0:32], ident[0:32, 0:32], ident[0:32, 0:32]
            )

            # ---- persistent per-stream tiles (memsets on GPSIMD) ----
            streams = [("n", 0, x_n), ("n", 1, x_n), ("c", 0, x_c), ("c", 1, x_c)]
            xt_hi, xt_lo, dct_sb = {}, {}, {}
            for sig, r, _ in streams:
                key = (sig, r)
                xt_hi[key] = pers.tile(
                    [128, NCH], F32, tag=f"xth_{sig}{r}", name=f"xth_{sig}{r}"
                )
                xt_lo[key] = pers.tile(
                    [32, NCH], F32, tag=f"xtl_{sig}{r}", name=f"xtl_{sig}{r}"
                )
                if sig == "n":
                    dct_sb[key] = pers.tile(
                        [128, T], F32, tag=f"dct_{sig}{r}", name=f"dct_{sig}{r}"
                    )
                    # rows 96:100 are rewritten by the clip; row 100 is the
                    # ones-row feeding the folded-in MLP bias
                    nc.gpsimd.memset(dct_sb[key][96:128, :], 1.0)
                else:
                    dct_sb[key] = pers.tile(
                        [NC, T], F32, tag=f"dct_{sig}{r}", name=f"dct_{sig}{r}"
                    )

            h_sb, od_sb = {}, {}
            for r in range(RPC):
                for layer in (0, 1):
                    t_ = pers.tile(
                        [128, T], F32, tag=f"h{layer}_{r}", name=f"h{layer}_{r}"
                    )
                    nc.gpsimd.memset(t_[96:128, :], 1.0)
                    h_sb[(r, layer)] = t_
                # od padded with a zero column on both ends so the idct
                # overlap-add shift never indexes out of range
                od_sb[r] = pers.tile([NC, T + 2], F32, tag=f"od_{r}", name=f"od{r}")
                # zero the pad columns on ACT so od_sb has a single writer
                # engine (ACT also writes the tanh output)
                nc.scalar.memzero(od_sb[r][:, 0:1])
                nc.scalar.memzero(od_sb[r][:, T + 1 : T + 2])

            # pre-sync PE and DVE on GPSIMD's memset ticks (1 wait each)
            gsync = pers.tile([32, 32], F32, tag="gsync", name="gsync")
            nc.gpsimd.memset(gsync[:], 0.0)
            pdum2 = ptr.tile([32, 500], F32, tag="trl", name="pdum2")
            nc.tensor.transpose(pdum2[0:32, 0:32], gsync[:], ident[0:32, 0:32])
            dve_scr = pers.tile([32, 32], F32, tag="dve_scr", name="dve_scr")
            nc.vector.tensor_copy(dve_scr[:], gsync[:])

            # ---- phase 1: load + transpose + DCT + clip, per stream ----
            for sig, r, xd in streams:
                key = (sig, r)
                # input loads: fresh slot per DMA (16 bufs, no reuse) so
                # each DMA carries zero sync waits; 2 time-chunks per DMA
                its = []
                for q in range(4):
                    it = inpool.tile([TCH, 2 * HOP], F32, tag="in", name="in_t")
                    nc.sync.dma_start(
                        it.rearrange("p (k s) -> p k s", k=2),
                        xd[r, 2 * q * TCH : (2 * q + 2) * TCH, :].rearrange(
                            "(k p) s -> p k s", k=2
                        ),
                    )
                    its.append(it)
                for half in range(2):
                    trh = ptr.tile([128, 4 * TCH], F32, tag="trh", name="trh")
                    trl = ptr.tile([32, 4 * TCH], F32, tag="trl", name="trl")
                    for kk in range(4):
                        k = 4 * half + kk
                        it = its[k // 2]
                        j = (k % 2) * HOP
                        nc.tensor.transpose(
                            trh[:, kk * TCH : (kk + 1) * TCH],
                            it[:, j : j + 128],
                            ident[:TCH, :TCH],
                        )
                        nc.tensor.transpose(
                            trl[:, kk * TCH : (kk + 1) * TCH],
                            it[:, j + 128 : j + HOP],
                            ident[:TCH, :TCH],
                        )
                    # single-engine (DVE) PSUM->SBUF copies
                    c0 = half * 4 * TCH
                    nc.vector.tensor_copy(
                        xt_hi[key][:, c0 : c0 + 4 * TCH], trh[:]
                    )
                    nc.vector.tensor_copy(
                        xt_lo[key][:, c0 : c0 + 4 * TCH], trl[:]
                    )

                for c0, w in ((0, 500), (500, T - 500)):
                    ps = pmm.tile([NC, 500], F32, tag="mm", name="ps_dct")
                    nc.tensor.matmul(
                        ps[:, :w], cp[:, _CHI0 : _CHI0 + NC],
                        xt_hi[key][:, c0 : c0 + w],
                        start=True, stop=False,
                    )
                    nc.tensor.matmul(
                        ps[:, :w], cp[0:32, _CLO0 : _CLO0 + NC],
                        xt_lo[key][:, c0 : c0 + w],
                        start=False, stop=False,
                    )
                    nc.tensor.matmul(
                        ps[:, :w], cp[:, _CHI0 + NC : _CHI0 + 2 * NC],
                        xt_hi[key][:, c0 + 1 : c0 + 1 + w],
                        start=False, stop=False,
                    )
                    nc.tensor.matmul(
                        ps[:, :w], cp[0:32, _CLO0 + NC : _CLO0 + 2 * NC],
                        xt_lo[key][:, c0 + 1 : c0 + 1 + w],
                        start=False, stop=True,
                    )
                    # clip to [-1, 1] while evicting PSUM -> SBUF
                    nc.vector.tensor_scalar(
                        out=dct_sb[key][0:NC, c0 : c0 + w],
                        in0=ps[:NC, :w],
                        scalar1=-1.0,
                        scalar2=1.0,
                        op0=mybir.AluOpType.max,
                        op1=mybir.AluOpType.min,
                    )
                if sig == "c":
                    nc.sync.dma_start(cd_out[r], dct_sb[key][:])

            # ---- phase 2: MLP over noisy streams ----
            for r in range(RPC):
                acts = dct_sb[("n", r)]
                for layer in range(3):
                    lhs = cp[0 : NC + 1, _WPK0 + layer * NC : _WPK0 + (layer + 1) * NC]
                    for c0, w in ((0, 500), (500, T - 500)):
                        ph = pmm.tile([NC, 500], F32, tag="mm", name="ps_mlp")
                        nc.tensor.matmul(
                            ph[:, :w], lhs, acts[0 : NC + 1, c0 : c0 + w],
                            start=True, stop=True,
                        )
                        if layer < 2:
                            p = (p1, p2)[layer]
                            tmp = evpool.tile([NC, 500], F32, tag="ptmp", name="ptmp")
                            nc.vector.tensor_scalar_mul(tmp[:, :w], ph[:NC, :w], float(p))
                            nc.vector.tensor_tensor(
                                out=h_sb[(r, layer)][0:NC, c0 : c0 + w],
                                in0=ph[:NC, :w],
                                in1=tmp[:, :w],
                                op=pre_op[layer],
                            )
                        else:
                            nc.scalar.activation(
                                od_sb[r][:, 1 + c0 : 1 + c0 + w],
                                ph[:NC, :w],
                                mybir.ActivationFunctionType.Tanh,
                            )
                    acts = h_sb[(r, layer)] if layer < 2 else None
                nc.sync.dma_start(od_out[r], od_sb[r][:, 1 : T + 1])

            # ---- phase 3: inverse DCT + overlap-add ----
            # rows interleaved so PSUM-slot reuse waits stay one-per-matmul
            for k in range(NTCH):
                for r in range(RPC):
                    od = od_sb[r]
                    c0 = k * TCH
                    pso = po.tile([TCH, HOP], F32, tag="idct", name="ps_o")
                    # out[c, s] = od[c].dctT[:, s] + od[c-1].dctT[:, 160+s]
                    nc.tensor.matmul(
                        pso[:], od[:, 1 + c0 : 1 + c0 + TCH],
                        cp[0:NC, _DCT0 : _DCT0 + HOP],
                        start=True, stop=False,
                    )
                    nc.tensor.matmul(
                        pso[:], od[:, c0 : c0 + TCH],
                        cp[0:NC, _DCT0 + HOP : _DCT0 + WIN],
                        start=False, stop=True,
                    )
                    ev = sppool.tile([TCH, HOP], F32, tag="spev", name="spev")
                    nc.vector.tensor_copy(ev[:], pso[:])
                    nc.sync.dma_start(sp_out[r, c0 : c0 + TCH, :], ev[:])

    nc.finalize()
    return nc


_CACHE = {}


def _pack_weights(W, bvec):
    wpk = np.zeros((NC + 1, 3 * NC), np.float32)
    for i in range(3):
        wpk[:NC, i * NC : (i + 1) * NC] = W[i].T
        wpk[NC, i * NC : (i + 1) * NC] = bvec[i]
    return wpk


def kernel(**inputs) -> tuple:
    noisy = np.ascontiguousarray(np.asarray(inputs["noisy"], dtype=np.float32))
    clean = np.ascontiguousarray(np.asarray(inputs["clean"], dtype=np.float32))
    W = [np.asarray(inputs[f"W{i}"], dtype=np.float32) for i in (1, 2, 3)]
    bvec = [np.asarray(inputs[f"b{i}"], dtype=np.float32) for i in (1, 2, 3)]
    p1 = float(np.asarray(inputs["p1"]))
    p2 = float(np.asarray(inputs["p2"]))

    chi, clo, dctT = _host_constants()
    cpack = _const_pack(chi, clo, dctT, _pack_weights(W, bvec))

    key = (p1, p2)
    if key not in _CACHE:
        _CACHE[key] = _build(p1, p2)
    nc = _CACHE[key]

    in_maps = []
    for c in range(NCORES):
        rows = slice(c * RPC, (c + 1) * RPC)
        in_maps.append(
            dict(
                noisy=noisy[rows].reshape(RPC, NCH, HOP),
                clean=clean[rows].reshape(RPC, NCH, HOP),
                cpack=cpack,
            )
        )

    res = run_bass_kernel_spmd(nc, in_maps, list(range(NCORES)))
    od = np.concatenate([r["od"].transpose(0, 2, 1) for r in res.results], axis=0)
    cd = np.concatenate([r["cd"].transpose(0, 2, 1) for r in res.results], axis=0)
    sp = np.concatenate([r["sp"].reshape(RPC, L) for r in res.results], axis=0)
    return (
        np.ascontiguousarray(od, np.float32),
        np.ascontiguousarray(cd, np.float32),
        np.ascontiguousarray(sp, np.float32),
    )
